# revision 3
# baseline (speedup 1.0000x reference)
"""Conv4d (Strang rearrange) Trainium2 kernel — banded-weight formulation, v13.

Sharding: 8 cores = batch(4) x H-half(2). Each core computes the full
(D1=32, D2=32) conv for its batch sample and 16 output H rows.

Matmul formulation (per output D1-row u, one PSUM bank):
  Data lanes = cin(4) x kh(2) x vl(10) = 80 + ones-lane (bias via matmul)
  + 15 zero lanes = 96. All 3 kv taps live inside the banded stationary
  W[(cin,kh,vl), (vo,cout)]; round (ku=1,kw=0) carries the bias against
  the ones-lane, so PSUM->SBUF is a plain DVE tensor_copy.
  M = 32 = vo(8) x cout(4); 4 v-blocks run as concurrent column tiles
  (tile_position=(0,32b)); 6 rounds (ku x kw) per u.

Port-balanced parity split (the v3 key change): SBUF AXI ports pair
partitions {x, x+32}; partitions 0-63 ride even ports, 64-127 odd.
A 96-partition load at [0:96] overloads even ports 2:1. So even rows'
lanes live at partitions [0:96] and odd rows' at [32:128] (stationary
band shifted down 32 rows for odd parity) -> every port carries the
same load bytes, and pad memsets stay 32-aligned ([96:128] even slots,
[0:32] odd slots). Loads sustain ~342 GB/s instead of ~255.

Schedule: all z loads on sync's HWDGE ring (FIFO => in-order arrival),
first rows as singles split sync/gpsimd for latency, then same-parity
pair chunks (768KB). Stores (2 output rows, 256KB) on scalar's ring
with per-slot-pair semaphores (completion increments from concurrent
stores interleave across SDMA engines, so one shared counter is racy).
8 warmup matmuls on garbage (overwritten via start=True) keep the PE
HAM clock at 2.4GHz through startup; DVE does PSUM->SBUF fp16 copies.
"""

from contextlib import ExitStack

import ml_dtypes
import numpy as np

import concourse.bass as bass
from concourse import bacc, mybir
from concourse.bass_utils import run_bass_kernel_spmd

F16 = mybir.dt.float16
BF16 = mybir.dt.bfloat16
F32 = mybir.dt.float32
F8E3 = mybir.dt.float8e3
U32 = mybir.dt.uint32

B, CIN, COUT = 4, 4, 4
D1, D2, H, W = 32, 32, 64, 64
U = D1
IL, J = 16, 32
NB = 4
VL = 10
KD = CIN * 2 * VL  # 80 data lanes
KB = 96  # 80 data + ones lane at 80 + zero lanes 81..95
K = 128
NCORES = 8
NZ, NPS, NOUT = 16, 8, 16
NWARM = 14
BIAS_RI = 2  # round (ku=1, kw=0) carries the bias row

# Ring slot map: even rows -> slots 0..7, odd rows -> slots 8..15, so
# same-parity multi-row chunks are contiguous slot ranges. zall rows are
# parity-permuted in DRAM (even rows first) so chunks are also contiguous
# on the DRAM side.
def _slot(r):
    return (r // 2) % 8 + 8 * (r % 2)


def _zpos(r):
    """Row index inside the parity-permuted zall."""
    return r // 2 + 16 * (r % 2)


# z DMA chunks: one row per DMA, all on sync's ring (FIFO, in-order,
# ~1.2us latency each). Rows >= 16 reuse ring slots: row r's slot held
# row r-16, last read at u = r-15 => gate mm >= r-14.
ZCHUNKS = [((r,), None if r < 16 else r - 14) for r in range(U)]
GPS_CHUNKS = ()

_DMA_OF = {}
for _d, (_rows, _) in enumerate(ZCHUNKS):
    for _r in _rows:
        _DMA_OF[_r] = _d

# memset slot -> (sem index, count) via per-engine ordered lists
MS_VEC = [0, 8, 1, 9, 2, 10]
MS_SCA = [3, 11, 4, 12]
MS_GPS = [5, 13, 6, 14, 7, 15]


def _host_weights(w, b):
    w = np.asarray(w, np.float32)
    wb = np.zeros((K, 2, 6, 32), np.float32)  # [row, parity, ri, m]
    for ku in range(3):
        for kw in range(2):
            ri = ku * 2 + kw
            for kh in range(2):
                for vl in range(VL):
                    for vo in range(8):
                        kv = vl - vo
                        if 0 <= kv <= 2:
                            wb[kh * VL + vl : KD : 2 * VL, 0, ri, vo * 4 : vo * 4 + 4] = w[
                                :, :, ku, kv, kh, kw
                            ].T
    wb[KD, 0, BIAS_RI, :] = np.tile(np.asarray(b, np.float32), 8)
    wb[32:, 1] = wb[:-32, 0]  # odd parity: band shifted down 32 rows
    return np.ascontiguousarray(wb.transpose(0, 2, 1, 3)).reshape(K, 12, 32).astype(
        ml_dtypes.bfloat16
    )  # [row, ri*2+parity, m]


def _host_shard(x):
    x = np.asarray(x, np.float32)
    shards = []
    vidx = (8 * np.arange(NB)[:, None] + np.arange(VL)[None, :]).reshape(-1)
    for core in range(NCORES):
        bb, hh = divmod(core, 2)
        xc = x[bb, :, :, :, 32 * hh : 32 * hh + 32, :]  # [cin, r, v, h, w]
        xp = np.pad(xc, ((0, 0), (0, 0), (1, 1), (0, 0), (0, 0)))
        arr = xp[:, :, vidx]
        arr = arr.reshape(CIN, U, NB, VL, IL, 2, J, 2)
        arr = arr.transpose(0, 5, 3, 1, 7, 2, 4, 6)  # cin, kh, vl, r, kw, b, i, j
        arr = np.ascontiguousarray(arr).reshape(KD, U, 2, NB, IL, J)
        perm = list(range(0, U, 2)) + list(range(1, U, 2))  # even rows first
        zall = np.zeros((KB, U, 2, NB, IL, J), np.float32)
        zall[:KD] = arr[:, perm]
        zall[KD] = 1.0
        shards.append({"zall": zall.astype(ml_dtypes.float8_e3m4)})
    return shards


def _build_program():
    nc = bacc.Bacc("TRN2", target_bir_lowering=False, debug=False)
    zall = nc.dram_tensor("zall", [KB, U, 2, NB, IL, J], F8E3, kind="ExternalInput").ap()
    wbd = nc.dram_tensor("wbd", [K, 12, 32], BF16, kind="ExternalInput").ap()
    ys = nc.dram_tensor("ys", [U // 2, 128, 2, IL, J], F16, kind="ExternalOutput").ap()

    with ExitStack() as ctx:
        ztr = ctx.enter_context(nc.sbuf_tensor("ztr", [K, NZ, 2, NB, IL, J], F8E3))
        wt = ctx.enter_context(nc.sbuf_tensor("wt", [K, 12, 32], BF16))
        otr = ctx.enter_context(nc.sbuf_tensor("otr", [128, NOUT, IL, J], F16))
        ps = [ctx.enter_context(nc.psum_tensor(f"ps{i}", [128, IL, J], F32)) for i in range(NPS)]
        sem_w = ctx.enter_context(nc.semaphore("sem_w"))
        sem_z = [ctx.enter_context(nc.semaphore(f"sem_z{d}")) for d in range(len(ZCHUNKS))]
        sem_mm = ctx.enter_context(nc.semaphore("sem_mm"))
        sem_cp = ctx.enter_context(nc.semaphore("sem_cp"))
        sem_stq = [ctx.enter_context(nc.semaphore(f"sem_stq{q}")) for q in range(4)]
        sem_msv = ctx.enter_context(nc.semaphore("sem_msv"))
        sem_msg = ctx.enter_context(nc.semaphore("sem_msg"))
        sem_mss = ctx.enter_context(nc.semaphore("sem_mss"))
        blk_ctx = nc.Block()
        block = blk_ctx.__enter__()

        def _ms_gate(s):
            if s in MS_VEC:
                return sem_msv, MS_VEC.index(s) + 1
            if s in MS_GPS:
                return sem_msg, MS_GPS.index(s) + 1
            return sem_mss, MS_SCA.index(s) + 1

        def _zdma(eng, d):
            rows, gate = ZCHUNKS[d]
            if gate is not None:
                eng.wait_ge(sem_mm, gate)
            r0 = rows[0]
            par = r0 % 2
            s0 = _slot(r0)
            p0 = _zpos(r0)
            n = len(rows)
            dst = ztr[32 * par : 32 * par + KB, s0 : s0 + n]
            src = zall[:, p0 : p0 + n]
            eng.dma_start(dst, src).then_inc(sem_z[d], 16)

        def _memset(eng_ns, s, sem):
            par = s // 8
            lo = 96 if par == 0 else 0
            ap = ztr[lo : lo + 32, s].bitcast(U32)
            eng_ns.memset(ap, 0).then_inc(sem)

        def _store(eng, p):
            eng.wait_ge(sem_cp, 2 * p + 2)
            s = (2 * p) % NOUT
            eng.dma_start(ys[p], otr[:, s : s + 2]).then_inc(sem_stq[p % 4], 16)

        @block.sync
        def _(sync):
            for d in range(len(ZCHUNKS)):
                _zdma(sync, d)
            for q in range(4):
                sync.wait_ge(sem_stq[q], 16 * (U // 8) + (16 if q >= 2 else 0))

        @block.gpsimd
        def _(gpsimd):
            for s in MS_GPS:
                _memset(nc.gpsimd, s, sem_msg)

        @block.scalar
        def _(scalar):
            scalar.dma_start(wt[:], wbd[:]).then_inc(sem_w, 16)
            for s in MS_SCA:
                par = s // 8
                lo = 96 if par == 0 else 0
                nc.scalar.memzero(ztr[lo : lo + 32, s]).then_inc(sem_mss)
            for p in range(U // 2 - 2):
                _store(scalar, p)
            for u in range(U - 4, U):
                scalar.wait_ge(sem_cp, u + 1)
                pp, e = divmod(u, 2)
                scalar.dma_start(ys[pp, :, e], otr[:, u % NOUT]).then_inc(
                    sem_stq[pp % 4], 16
                )

        @block.vector
        def _(vector):
            for s in MS_VEC:
                _memset(nc.vector, s, sem_msv)
            for u in range(U):
                vector.wait_ge(sem_mm, u + 1)
                if u >= NOUT:
                    p = (u - NOUT) // 2
                    vector.wait_ge(sem_stq[p % 4], 16 * (p // 4 + 1))
                nc.vector.tensor_copy(otr[:, u % NOUT], ps[u % NPS][:]).then_inc(sem_cp)

        @block.tensor
        def _(tensor):
            for _ in range(NWARM):
                nc.tensor.matmul(
                    ps[NPS - 1][:32, :, :],
                    wt[:, 0, :],
                    ztr[:, 0, 0, 0, :, :],
                    start=True,
                    stop=True,
                    skip_group_check=True,
                    tile_position=(0, 0),
                )
            tensor.wait_ge(sem_w, 16)
            waited = set()
            prev_gate = None

            def _gates_for(u):
                """Emit the semaphore waits needed before u's matmuls."""
                nonlocal prev_gate
                if u == 0:
                    for d in (_DMA_OF[0], _DMA_OF[1]):
                        tensor.wait_ge(sem_z[d], 16)
                        waited.add(d)
                    g = _ms_gate(_slot(0))
                    tensor.wait_ge(*g)
                d = _DMA_OF[min(u + 1, U - 1)]
                if d not in waited:
                    tensor.wait_ge(sem_z[d], 16)
                    waited.add(d)
                if u + 1 < 16:
                    g = _ms_gate(_slot(u + 1))
                    if g != prev_gate:
                        tensor.wait_ge(*g)
                        prev_gate = g
                if u >= NPS:
                    tensor.wait_ge(sem_cp, u - NPS + 1)

            _gates_for(0)
            for u in range(U):
                psg = ps[u % NPS]
                rounds = [
                    (ku, kw)
                    for ku in range(3)
                    if 0 <= u + ku - 1 < U
                    for kw in range(2)
                ]
                last = None
                for idx, (ku, kw) in enumerate(rounds):
                    if idx == len(rounds) - 1 and u + 1 < U:
                        # hide next-u gate processing under this wave
                        _gates_for(u + 1)
                    r = u + ku - 1
                    ri = (ku * 2 + kw) * 2 + (r % 2)
                    slot = _slot(r)
                    for b4 in range(NB):
                        last = nc.tensor.matmul(
                            psg[32 * b4 : 32 * b4 + 32, :, :],
                            wt[:, ri, :],
                            ztr[:, slot, kw, b4, :, :],
                            start=(idx == 0),
                            stop=(idx == len(rounds) - 1),
                            skip_group_check=True,
                            tile_position=(0, 32 * b4),
                        )
                last.then_inc(sem_mm)

        blk_ctx.__exit__(None, None, None)

    nc.compile()
    return nc


def _unshard(results):
    y = np.empty((B, COUT, D1, D2, H // 2, W // 2), np.float32)
    for core in range(NCORES):
        bb, hh = divmod(core, 2)
        arr = results[core]["ys"].astype(np.float32)  # [16, 128, 2, 16, 32]
        arr = arr.reshape(U // 2, 32, COUT, 2, IL, J)
        arr = arr.transpose(2, 0, 3, 1, 4, 5).reshape(COUT, U, D2, IL, J)
        y[bb, :, :, :, 16 * hh : 16 * hh + 16, :] = arr
    return y


TRACE = False
LAST_RESULT = [None]


def kernel(x, w, b, _cache={}):
    if "nc" not in _cache:
        _cache["nc"] = _build_program()
    nc = _cache["nc"]
    wbd = _host_weights(w, b)
    in_maps = [dict(sh, wbd=wbd) for sh in _host_shard(x)]
    res = run_bass_kernel_spmd(nc, in_maps, list(range(NCORES)), trace=TRACE)
    LAST_RESULT[0] = res
    return _unshard(res.results)


# revision 4
# speedup vs baseline: 1.0637x; 1.0637x over previous
"""Conv4d (Strang rearrange) Trainium2 kernel — banded-weight formulation, v17 (fp8 out).

Sharding: 8 cores = batch(4) x H-half(2). Each core computes the full
(D1=32, D2=32) conv for its batch sample and 16 output H rows.

Matmul formulation (per output D1-row u, one PSUM bank):
  Data lanes = cin(4) x kh(2) x vl(10) = 80 + ones-lane (bias via matmul)
  + 15 zero lanes = 96. All 3 kv taps live inside the banded stationary
  W[(cin,kh,vl), (vo,cout)]; round (ku=1,kw=0) carries the bias against
  the ones-lane, so PSUM->SBUF is a plain DVE tensor_copy.
  M = 32 = vo(8) x cout(4); 4 v-blocks run as concurrent column tiles
  (tile_position=(0,32b)); 6 rounds (ku x kw) per u.

Port-balanced parity split (the v3 key change): SBUF AXI ports pair
partitions {x, x+32}; partitions 0-63 ride even ports, 64-127 odd.
A 96-partition load at [0:96] overloads even ports 2:1. So even rows'
lanes live at partitions [0:96] and odd rows' at [32:128] (stationary
band shifted down 32 rows for odd parity) -> every port carries the
same load bytes, and pad memsets stay 32-aligned ([96:128] even slots,
[0:32] odd slots). Loads sustain ~342 GB/s instead of ~255.

Schedule: all z loads on sync's HWDGE ring (FIFO => in-order arrival),
first rows as singles split sync/gpsimd for latency, then same-parity
pair chunks (768KB). Stores (2 output rows, 256KB) on scalar's ring
with per-slot-pair semaphores (completion increments from concurrent
stores interleave across SDMA engines, so one shared counter is racy).
8 warmup matmuls on garbage (overwritten via start=True) keep the PE
HAM clock at 2.4GHz through startup; DVE does PSUM->SBUF fp16 copies.
"""

from contextlib import ExitStack

import ml_dtypes
import numpy as np

import concourse.bass as bass
from concourse import bacc, mybir
from concourse.bass_utils import run_bass_kernel_spmd

F16 = mybir.dt.float16
BF16 = mybir.dt.bfloat16
F32 = mybir.dt.float32
F8E3 = mybir.dt.float8e3
U32 = mybir.dt.uint32

B, CIN, COUT = 4, 4, 4
D1, D2, H, W = 32, 32, 64, 64
U = D1
IL, J = 16, 32
NB = 4
VL = 10
KD = CIN * 2 * VL  # 80 data lanes
KB = 96  # 80 data + ones lane at 80 + zero lanes 81..95
K = 128
NCORES = 8
NZ, NPS, NOUT = 16, 8, 16
NWARM = 14
BIAS_RI = 2  # round (ku=1, kw=0) carries the bias row

# Ring slot map: even rows -> slots 0..7, odd rows -> slots 8..15, so
# same-parity multi-row chunks are contiguous slot ranges. zall rows are
# parity-permuted in DRAM (even rows first) so chunks are also contiguous
# on the DRAM side.
def _slot(r):
    return (r // 2) % 8 + 8 * (r % 2)


def _zpos(r):
    """Row index inside the parity-permuted zall."""
    return r // 2 + 16 * (r % 2)


# z DMA chunks: one row per DMA, all on sync's ring (FIFO, in-order,
# ~1.2us latency each). Rows >= 16 reuse ring slots: row r's slot held
# row r-16, last read at u = r-15 => gate mm >= r-14.
ZCHUNKS = [((r,), None if r < 16 else r - 14) for r in range(U)]
GPS_CHUNKS = ()

_DMA_OF = {}
for _d, (_rows, _) in enumerate(ZCHUNKS):
    for _r in _rows:
        _DMA_OF[_r] = _d

# memset slot -> (sem index, count) via per-engine ordered lists
MS_VEC = [0, 8, 1, 9, 2, 10]
MS_SCA = [3, 11, 4, 12]
MS_GPS = [5, 13, 6, 14, 7, 15]


def _host_weights(w, b):
    w = np.asarray(w, np.float32)
    wb = np.zeros((K, 2, 6, 32), np.float32)  # [row, parity, ri, m]
    for ku in range(3):
        for kw in range(2):
            ri = ku * 2 + kw
            for kh in range(2):
                for vl in range(VL):
                    for vo in range(8):
                        kv = vl - vo
                        if 0 <= kv <= 2:
                            wb[kh * VL + vl : KD : 2 * VL, 0, ri, vo * 4 : vo * 4 + 4] = w[
                                :, :, ku, kv, kh, kw
                            ].T
    wb[KD, 0, BIAS_RI, :] = np.tile(np.asarray(b, np.float32), 8)
    wb[32:, 1] = wb[:-32, 0]  # odd parity: band shifted down 32 rows
    return np.ascontiguousarray(wb.transpose(0, 2, 1, 3)).reshape(K, 12, 32).astype(
        ml_dtypes.bfloat16
    )  # [row, ri*2+parity, m]


def _host_shard(x):
    x = np.asarray(x, np.float32)
    shards = []
    vidx = (8 * np.arange(NB)[:, None] + np.arange(VL)[None, :]).reshape(-1)
    for core in range(NCORES):
        bb, hh = divmod(core, 2)
        xc = x[bb, :, :, :, 32 * hh : 32 * hh + 32, :]  # [cin, r, v, h, w]
        xp = np.pad(xc, ((0, 0), (0, 0), (1, 1), (0, 0), (0, 0)))
        arr = xp[:, :, vidx]
        arr = arr.reshape(CIN, U, NB, VL, IL, 2, J, 2)
        arr = arr.transpose(0, 5, 3, 1, 7, 2, 4, 6)  # cin, kh, vl, r, kw, b, i, j
        arr = np.ascontiguousarray(arr).reshape(KD, U, 2, NB, IL, J)
        perm = list(range(0, U, 2)) + list(range(1, U, 2))  # even rows first
        zall = np.zeros((KB, U, 2, NB, IL, J), np.float32)
        zall[:KD] = arr[:, perm]
        zall[KD] = 1.0
        shards.append({"zall": zall.astype(ml_dtypes.float8_e3m4)})
    return shards


def _build_program():
    nc = bacc.Bacc("TRN2", target_bir_lowering=False, debug=False)
    zall = nc.dram_tensor("zall", [KB, U, 2, NB, IL, J], F8E3, kind="ExternalInput").ap()
    wbd = nc.dram_tensor("wbd", [K, 12, 32], BF16, kind="ExternalInput").ap()
    ys = nc.dram_tensor("ys", [U // 2, 128, 2, IL, J], F8E3, kind="ExternalOutput").ap()

    with ExitStack() as ctx:
        ztr = ctx.enter_context(nc.sbuf_tensor("ztr", [K, NZ, 2, NB, IL, J], F8E3))
        wt = ctx.enter_context(nc.sbuf_tensor("wt", [K, 12, 32], BF16))
        otr = ctx.enter_context(nc.sbuf_tensor("otr", [128, NOUT, IL, J], F8E3))
        ps = [ctx.enter_context(nc.psum_tensor(f"ps{i}", [128, IL, J], F32)) for i in range(NPS)]
        sem_w = ctx.enter_context(nc.semaphore("sem_w"))
        sem_z = [ctx.enter_context(nc.semaphore(f"sem_z{d}")) for d in range(len(ZCHUNKS))]
        sem_mm = ctx.enter_context(nc.semaphore("sem_mm"))
        sem_cp = ctx.enter_context(nc.semaphore("sem_cp"))
        sem_stq = [ctx.enter_context(nc.semaphore(f"sem_stq{q}")) for q in range(4)]
        sem_msv = ctx.enter_context(nc.semaphore("sem_msv"))
        sem_msg = ctx.enter_context(nc.semaphore("sem_msg"))
        sem_mss = ctx.enter_context(nc.semaphore("sem_mss"))
        blk_ctx = nc.Block()
        block = blk_ctx.__enter__()

        def _ms_gate(s):
            if s in MS_VEC:
                return sem_msv, MS_VEC.index(s) + 1
            if s in MS_GPS:
                return sem_msg, MS_GPS.index(s) + 1
            return sem_mss, MS_SCA.index(s) + 1

        def _zdma(eng, d):
            rows, gate = ZCHUNKS[d]
            if gate is not None:
                eng.wait_ge(sem_mm, gate)
            r0 = rows[0]
            par = r0 % 2
            s0 = _slot(r0)
            p0 = _zpos(r0)
            n = len(rows)
            dst = ztr[32 * par : 32 * par + KB, s0 : s0 + n]
            src = zall[:, p0 : p0 + n]
            eng.dma_start(dst, src).then_inc(sem_z[d], 16)

        def _memset(eng_ns, s, sem):
            par = s // 8
            lo = 96 if par == 0 else 0
            ap = ztr[lo : lo + 32, s].bitcast(U32)
            eng_ns.memset(ap, 0).then_inc(sem)

        def _store(eng, p):
            eng.wait_ge(sem_cp, 2 * p + 2)
            s = (2 * p) % NOUT
            eng.dma_start(ys[p], otr[:, s : s + 2]).then_inc(sem_stq[p % 4], 16)

        @block.sync
        def _(sync):
            for d in range(len(ZCHUNKS)):
                _zdma(sync, d)
            for q in range(4):
                sync.wait_ge(sem_stq[q], 16 * (U // 8) + (16 if q >= 2 else 0))

        @block.gpsimd
        def _(gpsimd):
            for s in MS_GPS:
                _memset(nc.gpsimd, s, sem_msg)

        @block.scalar
        def _(scalar):
            scalar.dma_start(wt[:], wbd[:]).then_inc(sem_w, 16)
            for s in MS_SCA:
                par = s // 8
                lo = 96 if par == 0 else 0
                nc.scalar.memzero(ztr[lo : lo + 32, s]).then_inc(sem_mss)
            for p in range(U // 2 - 2):
                _store(scalar, p)
            for u in range(U - 4, U):
                scalar.wait_ge(sem_cp, u + 1)
                pp, e = divmod(u, 2)
                scalar.dma_start(ys[pp, :, e], otr[:, u % NOUT]).then_inc(
                    sem_stq[pp % 4], 16
                )

        @block.vector
        def _(vector):
            for s in MS_VEC:
                _memset(nc.vector, s, sem_msv)
            for u in range(U):
                vector.wait_ge(sem_mm, u + 1)
                if u >= NOUT:
                    p = (u - NOUT) // 2
                    vector.wait_ge(sem_stq[p % 4], 16 * (p // 4 + 1))
                nc.vector.tensor_copy(otr[:, u % NOUT], ps[u % NPS][:]).then_inc(sem_cp)

        @block.tensor
        def _(tensor):
            for _ in range(NWARM):
                nc.tensor.matmul(
                    ps[NPS - 1][:32, :, :],
                    wt[:, 0, :],
                    ztr[:, 0, 0, 0, :, :],
                    start=True,
                    stop=True,
                    skip_group_check=True,
                    tile_position=(0, 0),
                )
            tensor.wait_ge(sem_w, 16)
            waited = set()
            prev_gate = None

            def _gates_for(u):
                """Emit the semaphore waits needed before u's matmuls."""
                nonlocal prev_gate
                if u == 0:
                    for d in (_DMA_OF[0], _DMA_OF[1]):
                        tensor.wait_ge(sem_z[d], 16)
                        waited.add(d)
                    g = _ms_gate(_slot(0))
                    tensor.wait_ge(*g)
                d = _DMA_OF[min(u + 1, U - 1)]
                if d not in waited:
                    tensor.wait_ge(sem_z[d], 16)
                    waited.add(d)
                if u + 1 < 16:
                    g = _ms_gate(_slot(u + 1))
                    if g != prev_gate:
                        tensor.wait_ge(*g)
                        prev_gate = g
                if u >= NPS:
                    tensor.wait_ge(sem_cp, u - NPS + 1)

            _gates_for(0)
            for u in range(U):
                psg = ps[u % NPS]
                rounds = [
                    (ku, kw)
                    for ku in range(3)
                    if 0 <= u + ku - 1 < U
                    for kw in range(2)
                ]
                last = None
                for idx, (ku, kw) in enumerate(rounds):
                    if idx == len(rounds) - 1 and u + 1 < U:
                        # hide next-u gate processing under this wave
                        _gates_for(u + 1)
                    r = u + ku - 1
                    ri = (ku * 2 + kw) * 2 + (r % 2)
                    slot = _slot(r)
                    for b4 in range(NB):
                        last = nc.tensor.matmul(
                            psg[32 * b4 : 32 * b4 + 32, :, :],
                            wt[:, ri, :],
                            ztr[:, slot, kw, b4, :, :],
                            start=(idx == 0),
                            stop=(idx == len(rounds) - 1),
                            skip_group_check=True,
                            tile_position=(0, 32 * b4),
                        )
                last.then_inc(sem_mm)

        blk_ctx.__exit__(None, None, None)

    nc.compile()
    return nc


def _unshard(results):
    y = np.empty((B, COUT, D1, D2, H // 2, W // 2), np.float32)
    for core in range(NCORES):
        bb, hh = divmod(core, 2)
        arr = results[core]["ys"].astype(np.float32)  # [16, 128, 2, 16, 32]
        arr = arr.reshape(U // 2, 32, COUT, 2, IL, J)
        arr = arr.transpose(2, 0, 3, 1, 4, 5).reshape(COUT, U, D2, IL, J)
        y[bb, :, :, :, 16 * hh : 16 * hh + 16, :] = arr
    return y


TRACE = False
LAST_RESULT = [None]


def kernel(x, w, b, _cache={}):
    if "nc" not in _cache:
        _cache["nc"] = _build_program()
    nc = _cache["nc"]
    wbd = _host_weights(w, b)
    in_maps = [dict(sh, wbd=wbd) for sh in _host_shard(x)]
    res = run_bass_kernel_spmd(nc, in_maps, list(range(NCORES)), trace=TRACE)
    LAST_RESULT[0] = res
    return _unshard(res.results)


# revision 5
# speedup vs baseline: 1.0695x; 1.0054x over previous
"""Conv4d (Strang rearrange) Trainium2 kernel — banded-weight formulation, v18 (fp8 out, parallel first rows).

Sharding: 8 cores = batch(4) x H-half(2). Each core computes the full
(D1=32, D2=32) conv for its batch sample and 16 output H rows.

Matmul formulation (per output D1-row u, one PSUM bank):
  Data lanes = cin(4) x kh(2) x vl(10) = 80 + ones-lane (bias via matmul)
  + 15 zero lanes = 96. All 3 kv taps live inside the banded stationary
  W[(cin,kh,vl), (vo,cout)]; round (ku=1,kw=0) carries the bias against
  the ones-lane, so PSUM->SBUF is a plain DVE tensor_copy.
  M = 32 = vo(8) x cout(4); 4 v-blocks run as concurrent column tiles
  (tile_position=(0,32b)); 6 rounds (ku x kw) per u.

Port-balanced parity split (the v3 key change): SBUF AXI ports pair
partitions {x, x+32}; partitions 0-63 ride even ports, 64-127 odd.
A 96-partition load at [0:96] overloads even ports 2:1. So even rows'
lanes live at partitions [0:96] and odd rows' at [32:128] (stationary
band shifted down 32 rows for odd parity) -> every port carries the
same load bytes, and pad memsets stay 32-aligned ([96:128] even slots,
[0:32] odd slots). Loads sustain ~342 GB/s instead of ~255.

Schedule: all z loads on sync's HWDGE ring (FIFO => in-order arrival),
first rows as singles split sync/gpsimd for latency, then same-parity
pair chunks (768KB). Stores (2 output rows, 256KB) on scalar's ring
with per-slot-pair semaphores (completion increments from concurrent
stores interleave across SDMA engines, so one shared counter is racy).
8 warmup matmuls on garbage (overwritten via start=True) keep the PE
HAM clock at 2.4GHz through startup; DVE does PSUM->SBUF fp16 copies.
"""

from contextlib import ExitStack

import ml_dtypes
import numpy as np

import concourse.bass as bass
from concourse import bacc, mybir
from concourse.bass_utils import run_bass_kernel_spmd

F16 = mybir.dt.float16
BF16 = mybir.dt.bfloat16
F32 = mybir.dt.float32
F8E3 = mybir.dt.float8e3
U32 = mybir.dt.uint32

B, CIN, COUT = 4, 4, 4
D1, D2, H, W = 32, 32, 64, 64
U = D1
IL, J = 16, 32
NB = 4
VL = 10
KD = CIN * 2 * VL  # 80 data lanes
KB = 96  # 80 data + ones lane at 80 + zero lanes 81..95
K = 128
NCORES = 8
NZ, NPS, NOUT = 16, 8, 16
NWARM = 14
BIAS_RI = 2  # round (ku=1, kw=0) carries the bias row

# Ring slot map: even rows -> slots 0..7, odd rows -> slots 8..15, so
# same-parity multi-row chunks are contiguous slot ranges. zall rows are
# parity-permuted in DRAM (even rows first) so chunks are also contiguous
# on the DRAM side.
def _slot(r):
    return (r // 2) % 8 + 8 * (r % 2)


def _zpos(r):
    """Row index inside the parity-permuted zall."""
    return r // 2 + 16 * (r % 2)


# z DMA chunks: one row per DMA, all on sync's ring (FIFO, in-order,
# ~1.2us latency each). Rows >= 16 reuse ring slots: row r's slot held
# row r-16, last read at u = r-15 => gate mm >= r-14.
ZCHUNKS = [((r,), None if r < 16 else r - 14) for r in range(U)]
GPS_CHUNKS = ()

_DMA_OF = {}
for _d, (_rows, _) in enumerate(ZCHUNKS):
    for _r in _rows:
        _DMA_OF[_r] = _d

# memset slot -> (sem index, count) via per-engine ordered lists
MS_VEC = [0, 8, 1, 9, 2, 10]
MS_SCA = [3, 11, 4, 12]
MS_GPS = [5, 13, 6, 14, 7, 15]


def _host_weights(w, b):
    w = np.asarray(w, np.float32)
    wb = np.zeros((K, 2, 6, 32), np.float32)  # [row, parity, ri, m]
    for ku in range(3):
        for kw in range(2):
            ri = ku * 2 + kw
            for kh in range(2):
                for vl in range(VL):
                    for vo in range(8):
                        kv = vl - vo
                        if 0 <= kv <= 2:
                            wb[kh * VL + vl : KD : 2 * VL, 0, ri, vo * 4 : vo * 4 + 4] = w[
                                :, :, ku, kv, kh, kw
                            ].T
    wb[KD, 0, BIAS_RI, :] = np.tile(np.asarray(b, np.float32), 8)
    wb[32:, 1] = wb[:-32, 0]  # odd parity: band shifted down 32 rows
    return np.ascontiguousarray(wb.transpose(0, 2, 1, 3)).reshape(K, 12, 32).astype(
        ml_dtypes.bfloat16
    )  # [row, ri*2+parity, m]


def _host_shard(x):
    x = np.asarray(x, np.float32)
    shards = []
    vidx = (8 * np.arange(NB)[:, None] + np.arange(VL)[None, :]).reshape(-1)
    for core in range(NCORES):
        bb, hh = divmod(core, 2)
        xc = x[bb, :, :, :, 32 * hh : 32 * hh + 32, :]  # [cin, r, v, h, w]
        xp = np.pad(xc, ((0, 0), (0, 0), (1, 1), (0, 0), (0, 0)))
        arr = xp[:, :, vidx]
        arr = arr.reshape(CIN, U, NB, VL, IL, 2, J, 2)
        arr = arr.transpose(0, 5, 3, 1, 7, 2, 4, 6)  # cin, kh, vl, r, kw, b, i, j
        arr = np.ascontiguousarray(arr).reshape(KD, U, 2, NB, IL, J)
        perm = list(range(0, U, 2)) + list(range(1, U, 2))  # even rows first
        zall = np.zeros((KB, U, 2, NB, IL, J), np.float32)
        zall[:KD] = arr[:, perm]
        zall[KD] = 1.0
        shards.append({"zall": zall.astype(ml_dtypes.float8_e3m4)})
    return shards


def _build_program():
    nc = bacc.Bacc("TRN2", target_bir_lowering=False, debug=False)
    zall = nc.dram_tensor("zall", [KB, U, 2, NB, IL, J], F8E3, kind="ExternalInput").ap()
    wbd = nc.dram_tensor("wbd", [K, 12, 32], BF16, kind="ExternalInput").ap()
    ys = nc.dram_tensor("ys", [U // 2, 128, 2, IL, J], F8E3, kind="ExternalOutput").ap()

    with ExitStack() as ctx:
        ztr = ctx.enter_context(nc.sbuf_tensor("ztr", [K, NZ, 2, NB, IL, J], F8E3))
        wt = ctx.enter_context(nc.sbuf_tensor("wt", [K, 12, 32], BF16))
        otr = ctx.enter_context(nc.sbuf_tensor("otr", [128, NOUT, IL, J], F8E3))
        ps = [ctx.enter_context(nc.psum_tensor(f"ps{i}", [128, IL, J], F32)) for i in range(NPS)]
        sem_w = ctx.enter_context(nc.semaphore("sem_w"))
        sem_z = [ctx.enter_context(nc.semaphore(f"sem_z{d}")) for d in range(len(ZCHUNKS))]
        sem_mm = ctx.enter_context(nc.semaphore("sem_mm"))
        sem_cp = ctx.enter_context(nc.semaphore("sem_cp"))
        sem_stq = [ctx.enter_context(nc.semaphore(f"sem_stq{q}")) for q in range(4)]
        sem_msv = ctx.enter_context(nc.semaphore("sem_msv"))
        sem_msg = ctx.enter_context(nc.semaphore("sem_msg"))
        sem_mss = ctx.enter_context(nc.semaphore("sem_mss"))
        blk_ctx = nc.Block()
        block = blk_ctx.__enter__()

        def _ms_gate(s):
            if s in MS_VEC:
                return sem_msv, MS_VEC.index(s) + 1
            if s in MS_GPS:
                return sem_msg, MS_GPS.index(s) + 1
            return sem_mss, MS_SCA.index(s) + 1

        def _zdma(eng, d):
            rows, gate = ZCHUNKS[d]
            if gate is not None:
                eng.wait_ge(sem_mm, gate)
            r0 = rows[0]
            par = r0 % 2
            s0 = _slot(r0)
            p0 = _zpos(r0)
            n = len(rows)
            dst = ztr[32 * par : 32 * par + KB, s0 : s0 + n]
            src = zall[:, p0 : p0 + n]
            eng.dma_start(dst, src).then_inc(sem_z[d], 16)

        def _memset(eng_ns, s, sem):
            par = s // 8
            lo = 96 if par == 0 else 0
            ap = ztr[lo : lo + 32, s].bitcast(U32)
            eng_ns.memset(ap, 0).then_inc(sem)

        def _store(eng, p):
            eng.wait_ge(sem_cp, 2 * p + 2)
            s = (2 * p) % NOUT
            eng.dma_start(ys[p], otr[:, s : s + 2]).then_inc(sem_stq[p % 4], 16)

        SCALAR_CHUNKS = (1,)  # row 1 rides scalar's idle HWDGE ring at startup

        @block.sync
        def _(sync):
            for d in range(len(ZCHUNKS)):
                if d not in SCALAR_CHUNKS:
                    _zdma(sync, d)
            for q in range(4):
                sync.wait_ge(sem_stq[q], 16 * (U // 8) + (16 if q >= 2 else 0))

        @block.gpsimd
        def _(gpsimd):
            for s in MS_GPS:
                _memset(nc.gpsimd, s, sem_msg)

        @block.scalar
        def _(scalar):
            scalar.dma_start(wt[:], wbd[:]).then_inc(sem_w, 16)
            _zdma(scalar, 1)
            for s in MS_SCA:
                par = s // 8
                lo = 96 if par == 0 else 0
                nc.scalar.memzero(ztr[lo : lo + 32, s]).then_inc(sem_mss)
            for p in range(U // 2 - 2):
                _store(scalar, p)
            for u in range(U - 4, U):
                scalar.wait_ge(sem_cp, u + 1)
                pp, e = divmod(u, 2)
                scalar.dma_start(ys[pp, :, e], otr[:, u % NOUT]).then_inc(
                    sem_stq[pp % 4], 16
                )

        @block.vector
        def _(vector):
            for s in MS_VEC:
                _memset(nc.vector, s, sem_msv)
            for u in range(U):
                vector.wait_ge(sem_mm, u + 1)
                if u >= NOUT:
                    p = (u - NOUT) // 2
                    vector.wait_ge(sem_stq[p % 4], 16 * (p // 4 + 1))
                nc.vector.tensor_copy(otr[:, u % NOUT], ps[u % NPS][:]).then_inc(sem_cp)

        @block.tensor
        def _(tensor):
            for _ in range(NWARM):
                nc.tensor.matmul(
                    ps[NPS - 1][:32, :, :],
                    wt[:, 0, :],
                    ztr[:, 0, 0, 0, :, :],
                    start=True,
                    stop=True,
                    skip_group_check=True,
                    tile_position=(0, 0),
                )
            tensor.wait_ge(sem_w, 16)
            waited = set()
            prev_gate = None

            def _gates_for(u):
                """Emit the semaphore waits needed before u's matmuls."""
                nonlocal prev_gate
                if u == 0:
                    for d in (_DMA_OF[0], _DMA_OF[1]):
                        tensor.wait_ge(sem_z[d], 16)
                        waited.add(d)
                    g = _ms_gate(_slot(0))
                    tensor.wait_ge(*g)
                d = _DMA_OF[min(u + 1, U - 1)]
                if d not in waited:
                    tensor.wait_ge(sem_z[d], 16)
                    waited.add(d)
                if u + 1 < 16:
                    g = _ms_gate(_slot(u + 1))
                    if g != prev_gate:
                        tensor.wait_ge(*g)
                        prev_gate = g
                if u >= NPS:
                    tensor.wait_ge(sem_cp, u - NPS + 1)

            _gates_for(0)
            for u in range(U):
                psg = ps[u % NPS]
                rounds = [
                    (ku, kw)
                    for ku in range(3)
                    if 0 <= u + ku - 1 < U
                    for kw in range(2)
                ]
                last = None
                for idx, (ku, kw) in enumerate(rounds):
                    if idx == len(rounds) - 1 and u + 1 < U:
                        # hide next-u gate processing under this wave
                        _gates_for(u + 1)
                    r = u + ku - 1
                    ri = (ku * 2 + kw) * 2 + (r % 2)
                    slot = _slot(r)
                    for b4 in range(NB):
                        last = nc.tensor.matmul(
                            psg[32 * b4 : 32 * b4 + 32, :, :],
                            wt[:, ri, :],
                            ztr[:, slot, kw, b4, :, :],
                            start=(idx == 0),
                            stop=(idx == len(rounds) - 1),
                            skip_group_check=True,
                            tile_position=(0, 32 * b4),
                        )
                last.then_inc(sem_mm)

        blk_ctx.__exit__(None, None, None)

    nc.compile()
    return nc


def _unshard(results):
    y = np.empty((B, COUT, D1, D2, H // 2, W // 2), np.float32)
    for core in range(NCORES):
        bb, hh = divmod(core, 2)
        arr = results[core]["ys"].astype(np.float32)  # [16, 128, 2, 16, 32]
        arr = arr.reshape(U // 2, 32, COUT, 2, IL, J)
        arr = arr.transpose(2, 0, 3, 1, 4, 5).reshape(COUT, U, D2, IL, J)
        y[bb, :, :, :, 16 * hh : 16 * hh + 16, :] = arr
    return y


TRACE = False
LAST_RESULT = [None]


def kernel(x, w, b, _cache={}):
    if "nc" not in _cache:
        _cache["nc"] = _build_program()
    nc = _cache["nc"]
    wbd = _host_weights(w, b)
    in_maps = [dict(sh, wbd=wbd) for sh in _host_shard(x)]
    res = run_bass_kernel_spmd(nc, in_maps, list(range(NCORES)), trace=TRACE)
    LAST_RESULT[0] = res
    return _unshard(res.results)
